# revision 5
# baseline (speedup 1.0000x reference)
"""VGAE (GCN encoder + edge scoring) Trainium2 kernel, 8 NeuronCores SPMD.

Pipeline (per core, nodes sharded 8x12544 padded):
  dense1 (redundant, full): p1 = dinv * (x @ W1.T)        -> p1_full[100352,256] (local HBM)
  agg1 (own dst tiles): gather p1[src] + one-hot matmul scatter-add; h = relu(dinv*sum + b1)
  dense2 per tile: p2 = dinv * (h @ [Wmu;Wls].T)          -> p2_own[12544,128]
  AllGather p2 -> p2_full[100352,128] (Shared)
  agg2: gather p2[src] + one-hot scatter; mu/logstd; z = mu + eps*exp(min(ls,10))
  AllGather z -> z_full[100352,64]
  scoring: gather z[u], z[v], rowwise dot, sigmoid -> s
All message-passing uses indirect DMA (128 rows / instruction).
Graph constants (degree norm, edge order/padding) precomputed on host, as in the
reference where GCN norm is cached across layers.
"""
import os
import sys
import types

sys.path.insert(0, '/opt/trn_rl_repo')
import numpy as np

# --- optional NTFF profile hook (exec-time measurement under axon) ---
def _install_ntff_hook():
    if 'antenv.axon_hooks' in sys.modules:
        return True
    _hook = [None]
    mod = types.ModuleType('antenv.axon_hooks')
    mod.set_axon_ntff_profile_hook = lambda h: _hook.__setitem__(0, h)
    mod.get_axon_ntff_profile_hook = lambda: _hook[0]
    sys.modules['antenv.axon_hooks'] = mod
    try:
        from trn_agent_boot.trn_boot import _ntff_profile_via_ctypes
        mod.set_axon_ntff_profile_hook(
            _ntff_profile_via_ctypes('/opt/axon/libaxon_pjrt.so'))
        return True
    except Exception:
        return False


_HAVE_HOOK = _install_ntff_hook()

from concourse import bass, bacc, mybir
import concourse.tile as tile
from concourse.bass_utils import run_bass_kernel_spmd
from concourse.masks import make_identity

F32 = mybir.dt.float32
I32 = mybir.dt.int32

NCORE = 8
N = 100000
NPAD = 100352            # 8 * 12544
NSHARD = 12544           # 98 tiles of 128
TLOC = 98                # tiles per core
TGLob = 784              # global tiles
F = 512
H = 256
DZ = 64
ET = 1000000
ET_SH = ET // NCORE      # 125000
NSB = (ET_SH + 127) // 128   # 977 scoring blocks
PADDST = 999.0

MAX_LOGSTD = 10.0


def _host_prep(x, edge_index, target_edge_index, W1, b1, W_mu, b_mu, W_ls, b_ls, eps):
    """All static graph preprocessing + input staging."""
    src = np.asarray(edge_index[0], dtype=np.int64).astype(np.int32)
    dst = np.asarray(edge_index[1], dtype=np.int64).astype(np.int32)
    # self-loops as ordinary messages
    loop = np.arange(N, dtype=np.int32)
    src = np.concatenate([src, loop])
    dst = np.concatenate([dst, loop])
    E = src.shape[0]

    # reference deg counts messages incl self-loop; src/dst already include loops
    deg = np.bincount(dst, minlength=NPAD).astype(np.float32)
    deg[deg == 0] = 1.0
    dinv = (1.0 / np.sqrt(deg)).astype(np.float32)

    # ---- edge grouping by (core, local tile), padded to shared block counts
    gtile = dst >> 7                       # dst // 128
    order = np.argsort(gtile, kind='stable')
    src_s = src[order]
    dst_s = dst[order]
    gt_s = gtile[order]
    cnt = np.bincount(gtile, minlength=TGLob).reshape(NCORE, TLOC)   # [8, 98]
    nblk = np.maximum(1, np.ceil(cnt.max(axis=0) / 128.0)).astype(np.int64)  # [98]
    NBLK = int(nblk.sum())
    blk_off = np.concatenate([[0], np.cumsum(nblk)[:-1]])            # [98]

    seg_start = np.concatenate([[0], np.cumsum(cnt.flatten())[:-1]])  # per gtile
    rank_in_seg = np.arange(E, dtype=np.int64) - np.repeat(seg_start, cnt.flatten())
    core_e = gt_s // TLOC
    tloc_e = gt_s % TLOC
    col_e = blk_off[tloc_e] + (rank_in_seg >> 7)
    part_e = rank_in_seg & 127

    esrc = np.zeros((NCORE, 128, NBLK), np.int32)
    edst = np.full((NCORE, 128, NBLK), PADDST, np.float32)
    esrc[core_e, part_e, col_e] = src_s
    edst[core_e, part_e, col_e] = (dst_s & 127).astype(np.float32)

    # ---- scoring edges: shard by position
    tu = np.asarray(target_edge_index[0], dtype=np.int64).astype(np.int32)
    tv = np.asarray(target_edge_index[1], dtype=np.int64).astype(np.int32)
    tu_c = np.zeros((NCORE, 128, NSB), np.int32)
    tv_c = np.zeros((NCORE, 128, NSB), np.int32)
    for c in range(NCORE):
        a = tu[c * ET_SH:(c + 1) * ET_SH]
        b = tv[c * ET_SH:(c + 1) * ET_SH]
        ap = np.zeros(NSB * 128, np.int32)
        bp = np.zeros(NSB * 128, np.int32)
        ap[:ET_SH] = a
        bp[:ET_SH] = b
        tu_c[c] = ap.reshape(NSB, 128).T
        tv_c[c] = bp.reshape(NSB, 128).T

    # ---- dense inputs
    x = np.asarray(x, dtype=np.float32)
    xT = np.zeros((F, NPAD), np.float32)
    xT[:, :N] = x.T
    w1t = np.ascontiguousarray(np.asarray(W1, np.float32).T)          # [512,256]
    wcatT = np.ascontiguousarray(
        np.concatenate([np.asarray(W_mu, np.float32),
                        np.asarray(W_ls, np.float32)], axis=0).T)     # [256,128]
    b1c = np.ascontiguousarray(np.asarray(b1, np.float32).reshape(2, 128).T)  # [128,2]
    bcat = np.concatenate([np.asarray(b_mu, np.float32),
                           np.asarray(b_ls, np.float32)])[:, None]    # [128,1]
    dinv_all = np.ascontiguousarray(dinv.reshape(TGLob, 128).T)       # [128,784]
    eps = np.asarray(eps, np.float32)
    epsP = np.zeros((NPAD, DZ), np.float32)
    epsP[:N] = eps

    in_maps = []
    for c in range(NCORE):
        epsT = np.ascontiguousarray(epsP[c * NSHARD:(c + 1) * NSHARD].T)  # [64,12544]
        dinv_own = np.ascontiguousarray(
            dinv_all[:, c * TLOC:(c + 1) * TLOC])                     # [128,98]
        in_maps.append({
            "xT": xT, "w1t": w1t, "wcatT": wcatT, "b1c": b1c, "bcat": bcat,
            "dinv_all": dinv_all, "dinv_own": dinv_own, "epsT": epsT,
            "esrc": np.ascontiguousarray(esrc[c]),
            "edst": np.ascontiguousarray(edst[c]),
            "tu": np.ascontiguousarray(tu_c[c]),
            "tv": np.ascontiguousarray(tv_c[c]),
        })
    return in_maps, nblk, NBLK


def _build(nblk, NBLK):
    nc = bacc.Bacc(None, num_devices=NCORE, target_bir_lowering=False)

    xT = nc.dram_tensor("xT", [F, NPAD], F32, kind="ExternalInput")
    w1t = nc.dram_tensor("w1t", [F, H], F32, kind="ExternalInput")
    wcatT = nc.dram_tensor("wcatT", [H, 128], F32, kind="ExternalInput")
    b1c = nc.dram_tensor("b1c", [128, 2], F32, kind="ExternalInput")
    bcat = nc.dram_tensor("bcat", [128, 1], F32, kind="ExternalInput")
    dinv_all = nc.dram_tensor("dinv_all", [128, TGLob], F32, kind="ExternalInput")
    dinv_own = nc.dram_tensor("dinv_own", [128, TLOC], F32, kind="ExternalInput")
    epsT = nc.dram_tensor("epsT", [DZ, NSHARD], F32, kind="ExternalInput")
    esrc = nc.dram_tensor("esrc", [128, NBLK], I32, kind="ExternalInput")
    edst = nc.dram_tensor("edst", [128, NBLK], F32, kind="ExternalInput")
    tu = nc.dram_tensor("tu", [128, NSB], I32, kind="ExternalInput")
    tv = nc.dram_tensor("tv", [128, NSB], I32, kind="ExternalInput")

    z_out = nc.dram_tensor("z_out", [NSHARD, DZ], F32, kind="ExternalOutput")
    s_out = nc.dram_tensor("s_out", [NSB * 128], F32, kind="ExternalOutput")

    p1_full = nc.dram_tensor("p1_full", [NPAD, H], F32)
    p2_own = nc.dram_tensor("p2_own", [NSHARD, 128], F32)
    p2_full = nc.dram_tensor("p2_full", [NPAD, 128], F32, addr_space="Shared")
    z_own = nc.dram_tensor("z_own", [NSHARD, DZ], F32)
    z_full = nc.dram_tensor("z_full", [NPAD, DZ], F32, addr_space="Shared")

    rg = [list(range(NCORE))]
    blk_off = np.concatenate([[0], np.cumsum(nblk)[:-1]]).astype(int)

    with tile.TileContext(nc) as tc:
        with (
            tc.tile_pool(name="cst", bufs=1) as cpool,
            tc.tile_pool(name="edg", bufs=1) as epool,
        ):
            ident = cpool.tile([128, 128], F32, tag="ident")
            make_identity(nc, ident[:])
            iota = cpool.tile([128, 128], F32, tag="iota")
            nc.gpsimd.iota(iota[:], pattern=[[1, 128]], base=0,
                           channel_multiplier=0,
                           allow_small_or_imprecise_dtypes=True)
            w1_sb = cpool.tile([128, 4 * H], F32, tag="w1")       # 4 kc chunks
            for kc in range(4):
                nc.sync.dma_start(out=w1_sb[:, kc * H:(kc + 1) * H],
                                  in_=w1t[kc * 128:(kc + 1) * 128, :])
            wcat_sb = cpool.tile([128, 256], F32, tag="wcat")     # 2 chunks
            for oc in range(2):
                nc.sync.dma_start(out=wcat_sb[:, oc * 128:(oc + 1) * 128],
                                  in_=wcatT[oc * 128:(oc + 1) * 128, :])
            b1_sb = cpool.tile([128, 2], F32, tag="b1")
            nc.sync.dma_start(out=b1_sb[:], in_=b1c[:])
            bcat_sb = cpool.tile([128, 1], F32, tag="bcat")
            nc.sync.dma_start(out=bcat_sb[:], in_=bcat[:])
            dinv_sb = cpool.tile([128, TGLob], F32, tag="dinva")
            nc.sync.dma_start(out=dinv_sb[:], in_=dinv_all[:])
            dinvo_sb = cpool.tile([128, TLOC], F32, tag="dinvo")
            nc.sync.dma_start(out=dinvo_sb[:], in_=dinv_own[:])
            c10 = cpool.tile([128, 128], F32, tag="c10")
            nc.vector.memset(c10[:], MAX_LOGSTD)
            esrc_sb = epool.tile([128, NBLK], I32, tag="esrc")
            nc.sync.dma_start(out=esrc_sb[:], in_=esrc[:])
            edst_sb = epool.tile([128, NBLK], F32, tag="edst")
            nc.sync.dma_start(out=edst_sb[:], in_=edst[:])

            # ---------------- dense1: p1_full = dinv * (x @ W1.T), all nodes
            with (
                tc.tile_pool(name="d1x", bufs=3) as xpool,
                tc.tile_pool(name="d1s", bufs=3) as spool,
                tc.tile_pool(name="d1o", bufs=3) as opool,
                tc.tile_pool(name="d1p", bufs=3, space="PSUM") as pspool,
                tc.tile_pool(name="d1t", bufs=4, space="PSUM") as trpool,
            ):
                for rc in range(NPAD // 512):
                    r0 = rc * 512
                    xt = xpool.tile([128, 4 * 512], F32, tag="xt")
                    for kc in range(4):
                        nc.sync.dma_start(
                            out=xt[:, kc * 512:(kc + 1) * 512],
                            in_=xT[kc * 128:(kc + 1) * 128, r0:r0 + 512])
                    t1sb = spool.tile([128, 2 * 512], F32, tag="t1sb")
                    for oc in range(2):
                        ps = pspool.tile([128, 512], F32, space="PSUM", tag="d1ps")
                        for kc in range(4):
                            nc.tensor.matmul(
                                ps[:],
                                lhsT=w1_sb[:, kc * H + oc * 128:kc * H + (oc + 1) * 128],
                                rhs=xt[:, kc * 512:(kc + 1) * 512],
                                start=(kc == 0), stop=(kc == 3))
                        nc.vector.tensor_copy(
                            out=t1sb[:, oc * 512:(oc + 1) * 512], in_=ps[:])
                    for sub in range(4):
                        gt = rc * 4 + sub
                        p1t = opool.tile([128, H], F32, tag="p1t")
                        for oc in range(2):
                            trp = trpool.tile([128, 128], F32, space="PSUM",
                                              tag="trps")
                            nc.tensor.transpose(
                                trp[:],
                                t1sb[:, oc * 512 + sub * 128:oc * 512 + (sub + 1) * 128],
                                ident[:])
                            nc.vector.tensor_tensor(
                                out=p1t[:, oc * 128:(oc + 1) * 128],
                                in0=trp[:],
                                in1=dinv_sb[:, gt:gt + 1].to_broadcast([128, 128]),
                                op=mybir.AluOpType.mult)
                        nc.sync.dma_start(
                            out=p1_full[gt * 128:(gt + 1) * 128, :], in_=p1t[:])

            # ---------------- agg1 + dense2 per own tile
            with (
                tc.tile_pool(name="a1g", bufs=6) as gpool,
                tc.tile_pool(name="a1s", bufs=6) as sspool,
                tc.tile_pool(name="a1h", bufs=3) as hpool,
                tc.tile_pool(name="a1o", bufs=3) as opool,
                tc.tile_pool(name="a1p", bufs=2, space="PSUM") as aggps,
                tc.tile_pool(name="a1t", bufs=2, space="PSUM") as trps2,
            ):
                for t in range(TLOC):
                    nb = int(nblk[t])
                    j0 = int(blk_off[t])
                    psum_h = aggps.tile([128, H], F32, space="PSUM", tag="aggps")
                    for b in range(nb):
                        j = j0 + b
                        G = gpool.tile([128, H], F32, tag="G")
                        nc.gpsimd.indirect_dma_start(
                            out=G[:], out_offset=None,
                            in_=p1_full[:],
                            in_offset=bass.IndirectOffsetOnAxis(
                                ap=esrc_sb[:, j:j + 1], axis=0))
                        S = sspool.tile([128, 128], F32, tag="S")
                        nc.vector.tensor_tensor(
                            out=S[:], in0=iota[:],
                            in1=edst_sb[:, j:j + 1].to_broadcast([128, 128]),
                            op=mybir.AluOpType.is_equal)
                        nc.tensor.matmul(psum_h[:], lhsT=S[:], rhs=G[:],
                                         start=(b == 0), stop=(b == nb - 1))
                    # h = relu(dinv*psum + b1) computed feat-major
                    hpre = hpool.tile([128, H], F32, tag="hpre")
                    nc.vector.tensor_tensor(
                        out=hpre[:], in0=psum_h[:],
                        in1=dinvo_sb[:, t:t + 1].to_broadcast([128, H]),
                        op=mybir.AluOpType.mult)
                    hT = hpool.tile([128, 256], F32, tag="hT")  # 2 chunks [128,128]
                    for oc in range(2):
                        trp = trps2.tile([128, 128], F32, space="PSUM", tag="tr2")
                        nc.tensor.transpose(
                            trp[:], hpre[:, oc * 128:(oc + 1) * 128], ident[:])
                        nc.scalar.activation(
                            out=hT[:, oc * 128:(oc + 1) * 128], in_=trp[:],
                            func=mybir.ActivationFunctionType.Relu,
                            bias=b1_sb[:, oc:oc + 1])
                    t2ps = trps2.tile([128, 128], F32, space="PSUM", tag="t2ps")
                    for oc in range(2):
                        nc.tensor.matmul(
                            t2ps[:],
                            lhsT=wcat_sb[:, oc * 128:(oc + 1) * 128],
                            rhs=hT[:, oc * 128:(oc + 1) * 128],
                            start=(oc == 0), stop=(oc == 1))
                    t2sb = hpool.tile([128, 128], F32, tag="t2sb")
                    nc.vector.tensor_copy(out=t2sb[:], in_=t2ps[:])
                    p2ps = trps2.tile([128, 128], F32, space="PSUM", tag="p2ps")
                    nc.tensor.transpose(p2ps[:], t2sb[:], ident[:])
                    p2t = opool.tile([128, 128], F32, tag="p2t")
                    nc.vector.tensor_tensor(
                        out=p2t[:], in0=p2ps[:],
                        in1=dinvo_sb[:, t:t + 1].to_broadcast([128, 128]),
                        op=mybir.AluOpType.mult)
                    nc.sync.dma_start(
                        out=p2_own[t * 128:(t + 1) * 128, :], in_=p2t[:])

            # ---------------- AllGather p2
            nc.gpsimd.collective_compute(
                "AllGather", mybir.AluOpType.bypass, replica_groups=rg,
                ins=[p2_own[:]], outs=[p2_full[:]])
            tc.strict_bb_all_engine_barrier()

            # ---------------- agg2 per own tile -> z
            with (
                tc.tile_pool(name="a2g", bufs=6) as gpool,
                tc.tile_pool(name="a2s", bufs=6) as sspool,
                tc.tile_pool(name="a2h", bufs=3) as hpool,
                tc.tile_pool(name="a2e", bufs=3) as epool2,
                tc.tile_pool(name="a2p", bufs=2, space="PSUM") as aggps,
                tc.tile_pool(name="a2t", bufs=2, space="PSUM") as trps3,
            ):
                for t in range(TLOC):
                    nb = int(nblk[t])
                    j0 = int(blk_off[t])
                    psum_a = aggps.tile([128, 128], F32, space="PSUM", tag="agg2ps")
                    for b in range(nb):
                        j = j0 + b
                        G = gpool.tile([128, 128], F32, tag="G2")
                        nc.gpsimd.indirect_dma_start(
                            out=G[:], out_offset=None,
                            in_=p2_full[:],
                            in_offset=bass.IndirectOffsetOnAxis(
                                ap=esrc_sb[:, j:j + 1], axis=0))
                        S = sspool.tile([128, 128], F32, tag="S2")
                        nc.vector.tensor_tensor(
                            out=S[:], in0=iota[:],
                            in1=edst_sb[:, j:j + 1].to_broadcast([128, 128]),
                            op=mybir.AluOpType.is_equal)
                        nc.tensor.matmul(psum_a[:], lhsT=S[:], rhs=G[:],
                                         start=(b == 0), stop=(b == nb - 1))
                    apre = hpool.tile([128, 128], F32, tag="apre")
                    nc.vector.tensor_tensor(
                        out=apre[:], in0=psum_a[:],
                        in1=dinvo_sb[:, t:t + 1].to_broadcast([128, 128]),
                        op=mybir.AluOpType.mult)
                    aTps = trps3.tile([128, 128], F32, space="PSUM", tag="aT")
                    nc.tensor.transpose(aTps[:], apre[:], ident[:])
                    aT = hpool.tile([128, 128], F32, tag="aTsb")  # [mu;ls] x nodes
                    nc.vector.tensor_tensor(
                        out=aT[:], in0=aTps[:],
                        in1=bcat_sb[:, 0:1].to_broadcast([128, 128]),
                        op=mybir.AluOpType.add)
                    # ls rows 64:128 -> std = exp(min(ls,10))
                    nc.vector.tensor_tensor(
                        out=aT[64:128, :], in0=aT[64:128, :], in1=c10[64:128, :],
                        op=mybir.AluOpType.min)
                    std = hpool.tile([64, 128], F32, tag="std")
                    nc.scalar.activation(
                        out=std[:], in_=aT[64:128, :],
                        func=mybir.ActivationFunctionType.Exp)
                    et = epool2.tile([64, 128], F32, tag="et")
                    nc.sync.dma_start(out=et[:],
                                      in_=epsT[:, t * 128:(t + 1) * 128])
                    zT = hpool.tile([64, 128], F32, tag="zT")
                    nc.vector.tensor_tensor(out=zT[:], in0=std[:], in1=et[:],
                                            op=mybir.AluOpType.mult)
                    nc.vector.tensor_tensor(out=zT[:], in0=zT[:], in1=aT[:64, :],
                                            op=mybir.AluOpType.add)
                    zps = trps3.tile([128, 64], F32, space="PSUM", tag="zps")
                    nc.tensor.matmul(zps[:], lhsT=zT[:], rhs=ident[:64, :64],
                                     is_transpose=True)
                    zt = hpool.tile([128, 64], F32, tag="zt")
                    nc.vector.tensor_copy(out=zt[:], in_=zps[:])
                    nc.sync.dma_start(out=z_own[t * 128:(t + 1) * 128, :],
                                      in_=zt[:])
                    nc.sync.dma_start(out=z_out[t * 128:(t + 1) * 128, :],
                                      in_=zt[:])

            # ---------------- AllGather z
            nc.gpsimd.collective_compute(
                "AllGather", mybir.AluOpType.bypass, replica_groups=rg,
                ins=[z_own[:]], outs=[z_full[:]])
            tc.strict_bb_all_engine_barrier()

            # ---------------- scoring
            with (
                tc.tile_pool(name="scg", bufs=8) as gpool,
                tc.tile_pool(name="scw", bufs=4) as wpool,
                tc.tile_pool(name="sco", bufs=2) as spool3,
            ):
                tu_sb = epool.tile([128, NSB], I32, tag="tu")
                nc.sync.dma_start(out=tu_sb[:], in_=tu[:])
                tv_sb = epool.tile([128, NSB], I32, tag="tv")
                nc.sync.dma_start(out=tv_sb[:], in_=tv[:])
                PK = 8
                for g0 in range(0, NSB, PK):
                    pack = spool3.tile([128, PK], F32, tag="pack")
                    gw = min(PK, NSB - g0)
                    for gi in range(gw):
                        g = g0 + gi
                        Gu = gpool.tile([128, DZ], F32, tag="Gu")
                        nc.gpsimd.indirect_dma_start(
                            out=Gu[:], out_offset=None, in_=z_full[:],
                            in_offset=bass.IndirectOffsetOnAxis(
                                ap=tu_sb[:, g:g + 1], axis=0))
                        Gv = gpool.tile([128, DZ], F32, tag="Gv")
                        nc.gpsimd.indirect_dma_start(
                            out=Gv[:], out_offset=None, in_=z_full[:],
                            in_offset=bass.IndirectOffsetOnAxis(
                                ap=tv_sb[:, g:g + 1], axis=0))
                        m = wpool.tile([128, DZ], F32, tag="m")
                        nc.vector.tensor_tensor(out=m[:], in0=Gu[:], in1=Gv[:],
                                                op=mybir.AluOpType.mult)
                        r = wpool.tile([128, 1], F32, tag="r")
                        nc.vector.reduce_sum(r[:], m[:],
                                             axis=mybir.AxisListType.X)
                        nc.scalar.activation(
                            out=pack[:, gi:gi + 1], in_=r[:],
                            func=mybir.ActivationFunctionType.Sigmoid)
                    nc.sync.dma_start(
                        out=s_out[g0 * 128:(g0 + gw) * 128].rearrange(
                            "(j p) -> p j", p=128),
                        in_=pack[:, :gw])

    nc.compile()
    return nc


_CACHE = {}


def kernel(x, edge_index, target_edge_index, W1, b1, W_mu, b_mu, W_ls, b_ls,
           eps_noise):
    in_maps, nblk, NBLK = _host_prep(x, edge_index, target_edge_index,
                                     W1, b1, W_mu, b_mu, W_ls, b_ls, eps_noise)
    key = (NBLK, tuple(nblk.tolist()))
    if key not in _CACHE:
        _CACHE[key] = _build(nblk, NBLK)
    nc = _CACHE[key]

    trace = _HAVE_HOOK and os.environ.get("KERNEL_NO_TRACE", "0") != "1"
    try:
        res = run_bass_kernel_spmd(nc, in_maps, list(range(NCORE)), trace=trace)
    except Exception:
        if not trace:
            raise
        res = run_bass_kernel_spmd(nc, in_maps, list(range(NCORE)), trace=False)
    kernel.last_exec_time_ns = getattr(res, "exec_time_ns", None)

    z = np.concatenate([res.results[c]["z_out"] for c in range(NCORE)],
                       axis=0)[:N]
    s = np.concatenate([res.results[c]["s_out"][:ET_SH]
                        for c in range(NCORE)])
    return z, s


kernel.last_exec_time_ns = None


# revision 6
# speedup vs baseline: 1.1887x; 1.1887x over previous
"""VGAE (GCN encoder + edge scoring) Trainium2 kernel, 8 NeuronCores SPMD.

Pipeline (per core, nodes degree-balanced-relabeled, sharded 8x12544):
  dense1 (own shard): p1 = dinv * (x @ W1.T) -> p1_own; AllGather -> p1_full
  agg1 (own dst tiles): indirect-gather p1[src] + one-hot matmul scatter-add;
        h = relu(dinv*sum + b1); p2 = dinv*(h @ [Wmu;Wls].T) -> p2_own
  AllGather p2 -> p2_full
  agg2: gather p2[src] + one-hot scatter; mu/logstd; z = mu + eps*exp(min(ls,10))
  AllGather z -> z_full
  scoring (edges sharded by u-owner): gather z[v] only; dot extracted via
        Zv @ zT_own matmul + one-hot mask + row-reduce; sigmoid.
Graph constants (degree norm, edge order, relabeling) precomputed on host, as in
the reference where GCN norm is cached across layers.
"""
import os
import sys
import types

sys.path.insert(0, '/opt/trn_rl_repo')
import numpy as np

# --- optional NTFF profile hook (exec-time measurement under axon) ---
def _install_ntff_hook():
    if 'antenv.axon_hooks' in sys.modules:
        return True
    _hook = [None]
    mod = types.ModuleType('antenv.axon_hooks')
    mod.set_axon_ntff_profile_hook = lambda h: _hook.__setitem__(0, h)
    mod.get_axon_ntff_profile_hook = lambda: _hook[0]
    sys.modules['antenv.axon_hooks'] = mod
    try:
        from trn_agent_boot.trn_boot import _ntff_profile_via_ctypes
        mod.set_axon_ntff_profile_hook(
            _ntff_profile_via_ctypes('/opt/axon/libaxon_pjrt.so'))
        return True
    except Exception:
        return False


_HAVE_HOOK = _install_ntff_hook()

from concourse import bass, bacc, mybir
import concourse.tile as tile
from concourse.bass_utils import run_bass_kernel_spmd
from concourse.masks import make_identity

F32 = mybir.dt.float32
I32 = mybir.dt.int32

NCORE = 8
N = 100000
NPAD = 100352            # 8 * 12544 = 784 * 128
NSHARD = 12544
TLOC = 98
TGLOB = 784
F = 512
H = 256
DZ = 64
ET = 1000000
PADDST = 999.0
MAX_LOGSTD = 10.0


def _group_by_tile(keys, payloads, minor):
    """Group items by global tile (keys>>7), pad per (core,tile) to the
    cross-core max block count. Returns per-core [128, NBLK] arrays + nblk."""
    gtile = keys >> 7
    order = np.argsort(gtile, kind='stable')
    gt_s = gtile[order]
    cnt = np.bincount(gtile, minlength=TGLOB).reshape(NCORE, TLOC)
    nblk = np.maximum(1, np.ceil(cnt.max(axis=0) / 128.0)).astype(np.int64)
    NBLK = int(nblk.sum())
    blk_off = np.concatenate([[0], np.cumsum(nblk)[:-1]])
    seg_start = np.concatenate([[0], np.cumsum(cnt.flatten())[:-1]])
    n = keys.shape[0]
    rank = np.arange(n, dtype=np.int64) - np.repeat(seg_start, cnt.flatten())
    core_e = gt_s // TLOC
    tl_e = gt_s % TLOC
    col_e = blk_off[tl_e] + (rank >> 7)
    part_e = rank & 127
    outs = []
    for arr, pad, dt in payloads:
        o = np.full((NCORE, 128, NBLK), pad, dt)
        o[core_e, part_e, col_e] = arr[order]
        outs.append(o)
    if minor:
        slot = np.full((NCORE, NBLK * 128), -1, np.int64)
        slot[core_e, col_e * 128 + part_e] = order
        outs.append(slot)
    return outs, nblk, NBLK


def _host_prep(x, edge_index, target_edge_index, W1, b1, W_mu, b_mu, W_ls, b_ls, eps):
    src0 = np.asarray(edge_index[0], dtype=np.int64).astype(np.int32)
    dst0 = np.asarray(edge_index[1], dtype=np.int64).astype(np.int32)
    loop = np.arange(N, dtype=np.int32)
    src0 = np.concatenate([src0, loop])
    dst0 = np.concatenate([dst0, loop])

    # degree-balanced node relabeling: sorted-by-degree round-robin over tiles
    deg_old = np.bincount(dst0, minlength=NPAD)
    order_d = np.argsort(-deg_old, kind='stable')
    i = np.arange(NPAD, dtype=np.int64)
    perm = np.empty(NPAD, np.int64)
    perm[order_d] = (i % TGLOB) * 128 + (i // TGLOB)      # old -> new
    inv = np.empty(NPAD, np.int64)
    inv[perm] = i                                          # new -> old

    src = perm[src0].astype(np.int32)
    dst = perm[dst0].astype(np.int32)

    deg = np.bincount(dst, minlength=NPAD).astype(np.float32)
    deg[deg == 0] = 1.0
    dinv = (1.0 / np.sqrt(deg)).astype(np.float32)

    (esrc, edst), nblk, NBLK = _group_by_tile(
        dst, [(src, 0, np.int32),
              ((dst & 127).astype(np.float32), PADDST, np.float32)],
        minor=False)

    # scoring edges sharded by u-owner
    tu = perm[np.asarray(target_edge_index[0], dtype=np.int64)].astype(np.int32)
    tv = perm[np.asarray(target_edge_index[1], dtype=np.int64)].astype(np.int32)
    (sv, su, sslot), nblkS, NSB2 = _group_by_tile(
        tu, [(tv, 0, np.int32),
             ((tu & 127).astype(np.float32), PADDST, np.float32)],
        minor=True)

    # dense inputs (relabeled)
    x = np.asarray(x, dtype=np.float32)
    xpad = np.zeros((NPAD, F), np.float32)
    xpad[:N] = x
    eps = np.asarray(eps, np.float32)
    epad = np.zeros((NPAD, DZ), np.float32)
    epad[:N] = eps
    w1t = np.ascontiguousarray(np.asarray(W1, np.float32).T)
    wcatT = np.ascontiguousarray(
        np.concatenate([np.asarray(W_mu, np.float32),
                        np.asarray(W_ls, np.float32)], axis=0).T)
    b1c = np.ascontiguousarray(np.asarray(b1, np.float32).reshape(2, 128).T)
    bcat = np.concatenate([np.asarray(b_mu, np.float32),
                           np.asarray(b_ls, np.float32)])[:, None]
    dinv_cols = np.ascontiguousarray(dinv.reshape(TGLOB, 128).T)   # [128,784]

    in_maps = []
    for c in range(NCORE):
        rows_old = inv[c * NSHARD:(c + 1) * NSHARD]
        xT_own = np.ascontiguousarray(xpad[rows_old].T)            # [512,12544]
        epsT = np.ascontiguousarray(epad[rows_old].T)              # [64,12544]
        dinv_own = np.ascontiguousarray(
            dinv_cols[:, c * TLOC:(c + 1) * TLOC])
        in_maps.append({
            "xT_own": xT_own, "w1t": w1t, "wcatT": wcatT, "b1c": b1c,
            "bcat": bcat, "dinv_own": dinv_own, "epsT": epsT,
            "esrc": np.ascontiguousarray(esrc[c]),
            "edst": np.ascontiguousarray(edst[c]),
            "su": np.ascontiguousarray(su[c]),
            "sv": np.ascontiguousarray(sv[c]),
        })
    meta = {"perm": perm, "sslot": sslot}
    return in_maps, nblk, NBLK, nblkS, NSB2, meta


def _build(nblk, NBLK, nblkS, NSB2):
    nc = bacc.Bacc(None, num_devices=NCORE, target_bir_lowering=False)

    xT_own = nc.dram_tensor("xT_own", [F, NSHARD], F32, kind="ExternalInput")
    w1t = nc.dram_tensor("w1t", [F, H], F32, kind="ExternalInput")
    wcatT = nc.dram_tensor("wcatT", [H, 128], F32, kind="ExternalInput")
    b1c = nc.dram_tensor("b1c", [128, 2], F32, kind="ExternalInput")
    bcat = nc.dram_tensor("bcat", [128, 1], F32, kind="ExternalInput")
    dinv_own = nc.dram_tensor("dinv_own", [128, TLOC], F32, kind="ExternalInput")
    epsT = nc.dram_tensor("epsT", [DZ, NSHARD], F32, kind="ExternalInput")
    esrc = nc.dram_tensor("esrc", [128, NBLK], I32, kind="ExternalInput")
    edst = nc.dram_tensor("edst", [128, NBLK], F32, kind="ExternalInput")
    su = nc.dram_tensor("su", [128, NSB2], F32, kind="ExternalInput")
    sv = nc.dram_tensor("sv", [128, NSB2], I32, kind="ExternalInput")

    z_out = nc.dram_tensor("z_out", [NSHARD, DZ], F32, kind="ExternalOutput")
    s_out = nc.dram_tensor("s_out", [NSB2 * 128], F32, kind="ExternalOutput")

    p1_own = nc.dram_tensor("p1_own", [NSHARD, H], F32)
    p1_full = nc.dram_tensor("p1_full", [NPAD, H], F32, addr_space="Shared")
    p2_own = nc.dram_tensor("p2_own", [NSHARD, 128], F32)
    p2_full = nc.dram_tensor("p2_full", [NPAD, 128], F32, addr_space="Shared")
    z_own = nc.dram_tensor("z_own", [NSHARD, DZ], F32)
    z_full = nc.dram_tensor("z_full", [NPAD, DZ], F32, addr_space="Shared")

    rg = [list(range(NCORE))]
    blk_off = np.concatenate([[0], np.cumsum(nblk)[:-1]]).astype(int)
    blk_offS = np.concatenate([[0], np.cumsum(nblkS)[:-1]]).astype(int)

    with tile.TileContext(nc) as tc:
        with (
            tc.tile_pool(name="cst", bufs=1) as cpool,
            tc.tile_pool(name="edg", bufs=1) as epool,
        ):
            ident = cpool.tile([128, 128], F32, tag="ident")
            make_identity(nc, ident[:])
            iota = cpool.tile([128, 128], F32, tag="iota")
            nc.gpsimd.iota(iota[:], pattern=[[1, 128]], base=0,
                           channel_multiplier=0,
                           allow_small_or_imprecise_dtypes=True)
            w1_sb = cpool.tile([128, 4 * H], F32, tag="w1")
            for kc in range(4):
                nc.sync.dma_start(out=w1_sb[:, kc * H:(kc + 1) * H],
                                  in_=w1t[kc * 128:(kc + 1) * 128, :])
            wcat_sb = cpool.tile([128, 256], F32, tag="wcat")
            for oc in range(2):
                nc.sync.dma_start(out=wcat_sb[:, oc * 128:(oc + 1) * 128],
                                  in_=wcatT[oc * 128:(oc + 1) * 128, :])
            b1_sb = cpool.tile([128, 2], F32, tag="b1")
            nc.sync.dma_start(out=b1_sb[:], in_=b1c[:])
            bcat_sb = cpool.tile([128, 1], F32, tag="bcat")
            nc.sync.dma_start(out=bcat_sb[:], in_=bcat[:])
            dinvo_sb = cpool.tile([128, TLOC], F32, tag="dinvo")
            nc.sync.dma_start(out=dinvo_sb[:], in_=dinv_own[:])
            c10 = cpool.tile([128, 128], F32, tag="c10")
            nc.vector.memset(c10[:], MAX_LOGSTD)
            zT_all = cpool.tile([64, NSHARD], F32, tag="zT_all")
            esrc_sb = epool.tile([128, NBLK], I32, tag="esrc")
            nc.sync.dma_start(out=esrc_sb[:], in_=esrc[:])
            edst_sb = epool.tile([128, NBLK], F32, tag="edst")
            nc.sync.dma_start(out=edst_sb[:], in_=edst[:])

            # ---------------- dense1 (own shard): p1_own = dinv*(x@W1.T)
            with (
                tc.tile_pool(name="d1x", bufs=3) as xpool,
                tc.tile_pool(name="d1s", bufs=3) as spool,
                tc.tile_pool(name="d1o", bufs=3) as opool,
                tc.tile_pool(name="d1p", bufs=3, space="PSUM") as pspool,
                tc.tile_pool(name="d1t", bufs=4, space="PSUM") as trpool,
            ):
                for r0 in range(0, NSHARD, 512):
                    rw = min(512, NSHARD - r0)
                    xt = xpool.tile([128, 4 * 512], F32, tag="xt")
                    for kc in range(4):
                        nc.sync.dma_start(
                            out=xt[:, kc * 512:kc * 512 + rw],
                            in_=xT_own[kc * 128:(kc + 1) * 128, r0:r0 + rw])
                    t1sb = spool.tile([128, 2 * 512], F32, tag="t1sb")
                    for oc in range(2):
                        ps = pspool.tile([128, 512], F32, space="PSUM", tag="d1ps")
                        for kc in range(4):
                            nc.tensor.matmul(
                                ps[:, :rw],
                                lhsT=w1_sb[:, kc * H + oc * 128:kc * H + (oc + 1) * 128],
                                rhs=xt[:, kc * 512:kc * 512 + rw],
                                start=(kc == 0), stop=(kc == 3))
                        nc.vector.tensor_copy(
                            out=t1sb[:, oc * 512:oc * 512 + rw], in_=ps[:, :rw])
                    for sub in range(rw // 128):
                        t = r0 // 128 + sub
                        p1t = opool.tile([128, H], F32, tag="p1t")
                        for oc in range(2):
                            trp = trpool.tile([128, 128], F32, space="PSUM",
                                              tag="trps")
                            nc.tensor.transpose(
                                trp[:],
                                t1sb[:, oc * 512 + sub * 128:oc * 512 + (sub + 1) * 128],
                                ident[:])
                            nc.vector.tensor_tensor(
                                out=p1t[:, oc * 128:(oc + 1) * 128],
                                in0=trp[:],
                                in1=dinvo_sb[:, t:t + 1].to_broadcast([128, 128]),
                                op=mybir.AluOpType.mult)
                        nc.sync.dma_start(
                            out=p1_own[t * 128:(t + 1) * 128, :], in_=p1t[:])

            nc.gpsimd.collective_compute(
                "AllGather", mybir.AluOpType.bypass, replica_groups=rg,
                ins=[p1_own[:]], outs=[p1_full[:]])
            tc.strict_bb_all_engine_barrier()

            # ---------------- agg1 + dense2 per own tile
            with (
                tc.tile_pool(name="a1g", bufs=8) as gpool,
                tc.tile_pool(name="a1s", bufs=8) as sspool,
                tc.tile_pool(name="a1h", bufs=3) as hpool,
                tc.tile_pool(name="a1o", bufs=3) as opool,
                tc.tile_pool(name="a1p", bufs=2, space="PSUM") as aggps,
                tc.tile_pool(name="a1t", bufs=2, space="PSUM") as trps2,
            ):
                for t in range(TLOC):
                    nb = int(nblk[t])
                    j0 = int(blk_off[t])
                    psum_h = aggps.tile([128, H], F32, space="PSUM", tag="aggps")
                    for b in range(nb):
                        j = j0 + b
                        G = gpool.tile([128, H], F32, tag="G")
                        nc.gpsimd.indirect_dma_start(
                            out=G[:], out_offset=None,
                            in_=p1_full[:],
                            in_offset=bass.IndirectOffsetOnAxis(
                                ap=esrc_sb[:, j:j + 1], axis=0))
                        S = sspool.tile([128, 128], F32, tag="S")
                        nc.vector.tensor_tensor(
                            out=S[:], in0=iota[:],
                            in1=edst_sb[:, j:j + 1].to_broadcast([128, 128]),
                            op=mybir.AluOpType.is_equal)
                        nc.tensor.matmul(psum_h[:], lhsT=S[:], rhs=G[:],
                                         start=(b == 0), stop=(b == nb - 1))
                    hpre = hpool.tile([128, H], F32, tag="hpre")
                    nc.vector.tensor_tensor(
                        out=hpre[:], in0=psum_h[:],
                        in1=dinvo_sb[:, t:t + 1].to_broadcast([128, H]),
                        op=mybir.AluOpType.mult)
                    hT = hpool.tile([128, 256], F32, tag="hT")
                    for oc in range(2):
                        trp = trps2.tile([128, 128], F32, space="PSUM", tag="tr2")
                        nc.tensor.transpose(
                            trp[:], hpre[:, oc * 128:(oc + 1) * 128], ident[:])
                        nc.scalar.activation(
                            out=hT[:, oc * 128:(oc + 1) * 128], in_=trp[:],
                            func=mybir.ActivationFunctionType.Relu,
                            bias=b1_sb[:, oc:oc + 1])
                    t2ps = trps2.tile([128, 128], F32, space="PSUM", tag="t2ps")
                    for oc in range(2):
                        nc.tensor.matmul(
                            t2ps[:],
                            lhsT=wcat_sb[:, oc * 128:(oc + 1) * 128],
                            rhs=hT[:, oc * 128:(oc + 1) * 128],
                            start=(oc == 0), stop=(oc == 1))
                    t2sb = hpool.tile([128, 128], F32, tag="t2sb")
                    nc.vector.tensor_copy(out=t2sb[:], in_=t2ps[:])
                    p2ps = trps2.tile([128, 128], F32, space="PSUM", tag="p2ps")
                    nc.tensor.transpose(p2ps[:], t2sb[:], ident[:])
                    p2t = opool.tile([128, 128], F32, tag="p2t")
                    nc.vector.tensor_tensor(
                        out=p2t[:], in0=p2ps[:],
                        in1=dinvo_sb[:, t:t + 1].to_broadcast([128, 128]),
                        op=mybir.AluOpType.mult)
                    nc.sync.dma_start(
                        out=p2_own[t * 128:(t + 1) * 128, :], in_=p2t[:])

            nc.gpsimd.collective_compute(
                "AllGather", mybir.AluOpType.bypass, replica_groups=rg,
                ins=[p2_own[:]], outs=[p2_full[:]])
            tc.strict_bb_all_engine_barrier()

            # ---------------- agg2 per own tile -> z
            with (
                tc.tile_pool(name="a2g", bufs=8) as gpool,
                tc.tile_pool(name="a2s", bufs=8) as sspool,
                tc.tile_pool(name="a2h", bufs=3) as hpool,
                tc.tile_pool(name="a2e", bufs=3) as epool2,
                tc.tile_pool(name="a2p", bufs=2, space="PSUM") as aggps,
                tc.tile_pool(name="a2t", bufs=2, space="PSUM") as trps3,
            ):
                for t in range(TLOC):
                    nb = int(nblk[t])
                    j0 = int(blk_off[t])
                    psum_a = aggps.tile([128, 128], F32, space="PSUM", tag="agg2ps")
                    for b in range(nb):
                        j = j0 + b
                        G = gpool.tile([128, 128], F32, tag="G2")
                        nc.gpsimd.indirect_dma_start(
                            out=G[:], out_offset=None,
                            in_=p2_full[:],
                            in_offset=bass.IndirectOffsetOnAxis(
                                ap=esrc_sb[:, j:j + 1], axis=0))
                        S = sspool.tile([128, 128], F32, tag="S2")
                        nc.vector.tensor_tensor(
                            out=S[:], in0=iota[:],
                            in1=edst_sb[:, j:j + 1].to_broadcast([128, 128]),
                            op=mybir.AluOpType.is_equal)
                        nc.tensor.matmul(psum_a[:], lhsT=S[:], rhs=G[:],
                                         start=(b == 0), stop=(b == nb - 1))
                    apre = hpool.tile([128, 128], F32, tag="apre")
                    nc.vector.tensor_tensor(
                        out=apre[:], in0=psum_a[:],
                        in1=dinvo_sb[:, t:t + 1].to_broadcast([128, 128]),
                        op=mybir.AluOpType.mult)
                    aTps = trps3.tile([128, 128], F32, space="PSUM", tag="aT")
                    nc.tensor.transpose(aTps[:], apre[:], ident[:])
                    aT = hpool.tile([128, 128], F32, tag="aTsb")
                    nc.vector.tensor_tensor(
                        out=aT[:], in0=aTps[:],
                        in1=bcat_sb[:, 0:1].to_broadcast([128, 128]),
                        op=mybir.AluOpType.add)
                    nc.vector.tensor_tensor(
                        out=aT[64:128, :], in0=aT[64:128, :], in1=c10[64:128, :],
                        op=mybir.AluOpType.min)
                    std = hpool.tile([64, 128], F32, tag="std")
                    nc.scalar.activation(
                        out=std[:], in_=aT[64:128, :],
                        func=mybir.ActivationFunctionType.Exp)
                    et = epool2.tile([64, 128], F32, tag="et")
                    nc.sync.dma_start(out=et[:],
                                      in_=epsT[:, t * 128:(t + 1) * 128])
                    zT = zT_all[:, t * 128:(t + 1) * 128]
                    nc.vector.tensor_tensor(out=zT, in0=std[:], in1=et[:],
                                            op=mybir.AluOpType.mult)
                    nc.vector.tensor_tensor(out=zT, in0=zT, in1=aT[:64, :],
                                            op=mybir.AluOpType.add)
                    zps = trps3.tile([128, 64], F32, space="PSUM", tag="zps")
                    nc.tensor.transpose(zps[:], zT, ident[:64, :64])
                    zt = hpool.tile([128, 64], F32, tag="zt")
                    nc.vector.tensor_copy(out=zt[:], in_=zps[:])
                    nc.sync.dma_start(out=z_own[t * 128:(t + 1) * 128, :],
                                      in_=zt[:])
                    nc.sync.dma_start(out=z_out[t * 128:(t + 1) * 128, :],
                                      in_=zt[:])

            nc.gpsimd.collective_compute(
                "AllGather", mybir.AluOpType.bypass, replica_groups=rg,
                ins=[z_own[:]], outs=[z_full[:]])
            tc.strict_bb_all_engine_barrier()

            # ---------------- scoring: s = sigmoid(z[u].z[v]), u in own shard
            with (
                tc.tile_pool(name="scg", bufs=8) as gpool,
                tc.tile_pool(name="scw", bufs=6) as wpool,
                tc.tile_pool(name="sco", bufs=2) as spool3,
                tc.tile_pool(name="scp", bufs=3, space="PSUM") as scps,
                tc.tile_pool(name="sct", bufs=3, space="PSUM") as sctr,
            ):
                su_sb = epool.tile([128, NSB2], F32, tag="su")
                nc.sync.dma_start(out=su_sb[:], in_=su[:])
                sv_sb = epool.tile([128, NSB2], I32, tag="sv")
                nc.sync.dma_start(out=sv_sb[:], in_=sv[:])
                PK = 8
                pack = None
                NSBTOT = int(nblkS.sum())
                for t in range(TLOC):
                    nbs = int(nblkS[t])
                    g0 = int(blk_offS[t])
                    for b in range(nbs):
                        g = g0 + b
                        if g % PK == 0:
                            pack = spool3.tile([128, PK], F32, tag="pack")
                        Gv = gpool.tile([128, DZ], F32, tag="Gv")
                        nc.gpsimd.indirect_dma_start(
                            out=Gv[:], out_offset=None, in_=z_full[:],
                            in_offset=bass.IndirectOffsetOnAxis(
                                ap=sv_sb[:, g:g + 1], axis=0))
                        vtp = sctr.tile([64, 128], F32, space="PSUM", tag="vtp")
                        nc.tensor.transpose(vtp[:], Gv[:], ident[:])
                        ZvT = wpool.tile([64, 128], F32, tag="ZvT")
                        nc.vector.tensor_copy(out=ZvT[:], in_=vtp[:])
                        Pps = scps.tile([128, 128], F32, space="PSUM", tag="Pps")
                        nc.tensor.matmul(
                            Pps[:], lhsT=ZvT[:],
                            rhs=zT_all[:, t * 128:(t + 1) * 128],
                            start=True, stop=True)
                        S = wpool.tile([128, 128], F32, tag="Sm")
                        nc.vector.tensor_tensor(
                            out=S[:], in0=iota[:],
                            in1=su_sb[:, g:g + 1].to_broadcast([128, 128]),
                            op=mybir.AluOpType.is_equal)
                        m = wpool.tile([128, 128], F32, tag="m")
                        nc.vector.tensor_tensor(out=m[:], in0=Pps[:], in1=S[:],
                                                op=mybir.AluOpType.mult)
                        r = wpool.tile([128, 1], F32, tag="r")
                        nc.vector.reduce_sum(r[:], m[:],
                                             axis=mybir.AxisListType.X)
                        nc.scalar.activation(
                            out=pack[:, g % PK:g % PK + 1], in_=r[:],
                            func=mybir.ActivationFunctionType.Sigmoid)
                        if (g % PK == PK - 1) or (g == NSBTOT - 1):
                            gs = (g // PK) * PK
                            gw = g - gs + 1
                            nc.sync.dma_start(
                                out=s_out[gs * 128:(gs + gw) * 128].rearrange(
                                    "(j p) -> p j", p=128),
                                in_=pack[:, :gw])

    nc.compile()
    return nc


_CACHE = {}


def kernel(x, edge_index, target_edge_index, W1, b1, W_mu, b_mu, W_ls, b_ls,
           eps_noise):
    in_maps, nblk, NBLK, nblkS, NSB2, meta = _host_prep(
        x, edge_index, target_edge_index, W1, b1, W_mu, b_mu, W_ls, b_ls,
        eps_noise)
    key = (NBLK, NSB2, tuple(nblk.tolist()), tuple(nblkS.tolist()))
    if key not in _CACHE:
        _CACHE[key] = _build(nblk, NBLK, nblkS, NSB2)
    nc = _CACHE[key]

    trace = _HAVE_HOOK and os.environ.get("KERNEL_NO_TRACE", "0") != "1"
    try:
        res = run_bass_kernel_spmd(nc, in_maps, list(range(NCORE)), trace=trace)
    except Exception:
        if not trace:
            raise
        res = run_bass_kernel_spmd(nc, in_maps, list(range(NCORE)), trace=False)
    kernel.last_exec_time_ns = getattr(res, "exec_time_ns", None)

    perm = meta["perm"]
    sslot = meta["sslot"]
    z_dev = np.concatenate([res.results[c]["z_out"] for c in range(NCORE)],
                           axis=0)
    z = z_dev[perm[:N]]
    s = np.empty(ET, np.float32)
    for c in range(NCORE):
        sl = sslot[c]
        valid = sl >= 0
        s[sl[valid]] = res.results[c]["s_out"][valid]
    return z, s


kernel.last_exec_time_ns = None


# revision 7
# speedup vs baseline: 1.2003x; 1.0097x over previous
"""VGAE (GCN encoder + edge scoring) Trainium2 kernel, 8 NeuronCores SPMD.

Pipeline (per core, nodes degree-balanced-relabeled, sharded 8x12544):
  dense1 (own shard): p1 = dinv * (x @ W1.T) -> p1_own; AllGather -> p1_full
  agg1 (own dst tiles): indirect-gather p1[src] + one-hot matmul scatter-add;
        h = relu(dinv*sum + b1); p2 = dinv*(h @ [Wmu;Wls].T) -> p2_own
  AllGather p2 -> p2_full
  agg2: gather p2[src] + one-hot scatter; mu/logstd; z = mu + eps*exp(min(ls,10))
  AllGather z -> z_full
  scoring (edges sharded by u-owner): gather z[v] only; dot extracted via
        Zv @ zT_own matmul + one-hot mask + row-reduce; sigmoid.
Graph constants (degree norm, edge order, relabeling) precomputed on host, as in
the reference where GCN norm is cached across layers.
"""
import os
import sys
import types

sys.path.insert(0, '/opt/trn_rl_repo')
import numpy as np

# --- optional NTFF profile hook (exec-time measurement under axon) ---
def _install_ntff_hook():
    if 'antenv.axon_hooks' in sys.modules:
        return True
    _hook = [None]
    mod = types.ModuleType('antenv.axon_hooks')
    mod.set_axon_ntff_profile_hook = lambda h: _hook.__setitem__(0, h)
    mod.get_axon_ntff_profile_hook = lambda: _hook[0]
    sys.modules['antenv.axon_hooks'] = mod
    try:
        from trn_agent_boot.trn_boot import _ntff_profile_via_ctypes
        mod.set_axon_ntff_profile_hook(
            _ntff_profile_via_ctypes('/opt/axon/libaxon_pjrt.so'))
        return True
    except Exception:
        return False


_HAVE_HOOK = _install_ntff_hook()

from concourse import bass, bacc, mybir
import concourse.tile as tile
from concourse.bass_utils import run_bass_kernel_spmd
from concourse.masks import make_identity

F32 = mybir.dt.float32
I32 = mybir.dt.int32

NCORE = 8
N = 100000
NPAD = 100352            # 8 * 12544 = 784 * 128
NSHARD = 12544
TLOC = 98
TGLOB = 784
F = 512
H = 256
DZ = 64
ET = 1000000
PADDST = 999.0
MAX_LOGSTD = 10.0


def _group_by_tile(keys, payloads, minor):
    """Group items by global tile (keys>>7), pad per (core,tile) to the
    cross-core max block count. Returns per-core [128, NBLK] arrays + nblk."""
    gtile = keys >> 7
    order = np.argsort(gtile, kind='stable')
    gt_s = gtile[order]
    cnt = np.bincount(gtile, minlength=TGLOB).reshape(NCORE, TLOC)
    nblk = np.maximum(1, np.ceil(cnt.max(axis=0) / 128.0)).astype(np.int64)
    NBLK = int(nblk.sum())
    blk_off = np.concatenate([[0], np.cumsum(nblk)[:-1]])
    seg_start = np.concatenate([[0], np.cumsum(cnt.flatten())[:-1]])
    n = keys.shape[0]
    rank = np.arange(n, dtype=np.int64) - np.repeat(seg_start, cnt.flatten())
    core_e = gt_s // TLOC
    tl_e = gt_s % TLOC
    col_e = blk_off[tl_e] + (rank >> 7)
    part_e = rank & 127
    outs = []
    for arr, pad, dt in payloads:
        o = np.full((NCORE, 128, NBLK), pad, dt)
        o[core_e, part_e, col_e] = arr[order]
        outs.append(o)
    if minor:
        slot = np.full((NCORE, NBLK * 128), -1, np.int64)
        slot[core_e, col_e * 128 + part_e] = order
        outs.append(slot)
    return outs, nblk, NBLK


def _host_prep(x, edge_index, target_edge_index, W1, b1, W_mu, b_mu, W_ls, b_ls, eps):
    src0 = np.asarray(edge_index[0], dtype=np.int64).astype(np.int32)
    dst0 = np.asarray(edge_index[1], dtype=np.int64).astype(np.int32)
    loop = np.arange(N, dtype=np.int32)
    src0 = np.concatenate([src0, loop])
    dst0 = np.concatenate([dst0, loop])

    # degree-balanced node relabeling: sorted-by-degree round-robin over tiles
    deg_old = np.bincount(dst0, minlength=NPAD)
    order_d = np.argsort(-deg_old, kind='stable')
    i = np.arange(NPAD, dtype=np.int64)
    perm = np.empty(NPAD, np.int64)
    perm[order_d] = (i % TGLOB) * 128 + (i // TGLOB)      # old -> new
    inv = np.empty(NPAD, np.int64)
    inv[perm] = i                                          # new -> old

    src = perm[src0].astype(np.int32)
    dst = perm[dst0].astype(np.int32)

    deg = np.bincount(dst, minlength=NPAD).astype(np.float32)
    deg[deg == 0] = 1.0
    dinv = (1.0 / np.sqrt(deg)).astype(np.float32)

    (esrc, edst), nblk, NBLK = _group_by_tile(
        dst, [(src, 0, np.int32),
              ((dst & 127).astype(np.float32), PADDST, np.float32)],
        minor=False)

    # scoring edges sharded by u-owner
    tu = perm[np.asarray(target_edge_index[0], dtype=np.int64)].astype(np.int32)
    tv = perm[np.asarray(target_edge_index[1], dtype=np.int64)].astype(np.int32)
    (sv, su, sslot), nblkS, NSB2 = _group_by_tile(
        tu, [(tv, 0, np.int32),
             ((tu & 127).astype(np.float32), PADDST, np.float32)],
        minor=True)

    # dense inputs (relabeled)
    x = np.asarray(x, dtype=np.float32)
    xpad = np.zeros((NPAD, F), np.float32)
    xpad[:N] = x
    eps = np.asarray(eps, np.float32)
    epad = np.zeros((NPAD, DZ), np.float32)
    epad[:N] = eps
    w1t = np.ascontiguousarray(np.asarray(W1, np.float32).T)
    wcatT = np.ascontiguousarray(
        np.concatenate([np.asarray(W_mu, np.float32),
                        np.asarray(W_ls, np.float32)], axis=0).T)
    b1c = np.ascontiguousarray(np.asarray(b1, np.float32).reshape(2, 128).T)
    bcat = np.concatenate([np.asarray(b_mu, np.float32),
                           np.asarray(b_ls, np.float32)])[:, None]
    dinv_cols = np.ascontiguousarray(dinv.reshape(TGLOB, 128).T)   # [128,784]

    in_maps = []
    for c in range(NCORE):
        rows_old = inv[c * NSHARD:(c + 1) * NSHARD]
        xT_own = np.ascontiguousarray(xpad[rows_old].T)            # [512,12544]
        epsT = np.ascontiguousarray(epad[rows_old].T)              # [64,12544]
        dinv_own = np.ascontiguousarray(
            dinv_cols[:, c * TLOC:(c + 1) * TLOC])
        in_maps.append({
            "xT_own": xT_own, "w1t": w1t, "wcatT": wcatT, "b1c": b1c,
            "bcat": bcat, "dinv_own": dinv_own, "epsT": epsT,
            "esrc": np.ascontiguousarray(esrc[c]),
            "edst": np.ascontiguousarray(edst[c]),
            "su": np.ascontiguousarray(su[c]),
            "sv": np.ascontiguousarray(sv[c]),
        })
    meta = {"perm": perm, "sslot": sslot}
    return in_maps, nblk, NBLK, nblkS, NSB2, meta


def _build(nblk, NBLK, nblkS, NSB2):
    nc = bacc.Bacc(None, num_devices=NCORE, target_bir_lowering=False)

    xT_own = nc.dram_tensor("xT_own", [F, NSHARD], F32, kind="ExternalInput")
    w1t = nc.dram_tensor("w1t", [F, H], F32, kind="ExternalInput")
    wcatT = nc.dram_tensor("wcatT", [H, 128], F32, kind="ExternalInput")
    b1c = nc.dram_tensor("b1c", [128, 2], F32, kind="ExternalInput")
    bcat = nc.dram_tensor("bcat", [128, 1], F32, kind="ExternalInput")
    dinv_own = nc.dram_tensor("dinv_own", [128, TLOC], F32, kind="ExternalInput")
    epsT = nc.dram_tensor("epsT", [DZ, NSHARD], F32, kind="ExternalInput")
    esrc = nc.dram_tensor("esrc", [128, NBLK], I32, kind="ExternalInput")
    edst = nc.dram_tensor("edst", [128, NBLK], F32, kind="ExternalInput")
    su = nc.dram_tensor("su", [128, NSB2], F32, kind="ExternalInput")
    sv = nc.dram_tensor("sv", [128, NSB2], I32, kind="ExternalInput")

    z_out = nc.dram_tensor("z_out", [NSHARD, DZ], F32, kind="ExternalOutput")
    s_out = nc.dram_tensor("s_out", [NSB2 * 128], F32, kind="ExternalOutput")

    p1_own = nc.dram_tensor("p1_own", [NSHARD, H], F32)
    p1_full = nc.dram_tensor("p1_full", [NPAD, H], F32, addr_space="Shared")
    p2_own = nc.dram_tensor("p2_own", [NSHARD, 128], F32)
    p2_full = nc.dram_tensor("p2_full", [NPAD, 128], F32, addr_space="Shared")
    z_own = nc.dram_tensor("z_own", [NSHARD, DZ], F32)
    z_full = nc.dram_tensor("z_full", [NPAD, DZ], F32, addr_space="Shared")

    rg = [list(range(NCORE))]
    blk_off = np.concatenate([[0], np.cumsum(nblk)[:-1]]).astype(int)
    blk_offS = np.concatenate([[0], np.cumsum(nblkS)[:-1]]).astype(int)

    with tile.TileContext(nc) as tc:
        with (
            tc.tile_pool(name="cst", bufs=1) as cpool,
            tc.tile_pool(name="edg", bufs=1) as epool,
        ):
            ident = cpool.tile([128, 128], F32, tag="ident")
            make_identity(nc, ident[:])
            iota = cpool.tile([128, 128], F32, tag="iota")
            nc.gpsimd.iota(iota[:], pattern=[[1, 128]], base=0,
                           channel_multiplier=0,
                           allow_small_or_imprecise_dtypes=True)
            w1_sb = cpool.tile([128, 4 * H], F32, tag="w1")
            for kc in range(4):
                nc.sync.dma_start(out=w1_sb[:, kc * H:(kc + 1) * H],
                                  in_=w1t[kc * 128:(kc + 1) * 128, :])
            wcat_sb = cpool.tile([128, 256], F32, tag="wcat")
            for oc in range(2):
                nc.sync.dma_start(out=wcat_sb[:, oc * 128:(oc + 1) * 128],
                                  in_=wcatT[oc * 128:(oc + 1) * 128, :])
            b1_sb = cpool.tile([128, 2], F32, tag="b1")
            nc.sync.dma_start(out=b1_sb[:], in_=b1c[:])
            bcat_sb = cpool.tile([128, 1], F32, tag="bcat")
            nc.sync.dma_start(out=bcat_sb[:], in_=bcat[:])
            dinvo_sb = cpool.tile([128, TLOC], F32, tag="dinvo")
            nc.sync.dma_start(out=dinvo_sb[:], in_=dinv_own[:])
            c10 = cpool.tile([128, 128], F32, tag="c10")
            nc.vector.memset(c10[:], MAX_LOGSTD)
            zT_all = cpool.tile([64, NSHARD], F32, tag="zT_all")
            esrc_sb = epool.tile([128, NBLK], I32, tag="esrc")
            nc.sync.dma_start(out=esrc_sb[:], in_=esrc[:])
            edst_sb = epool.tile([128, NBLK], F32, tag="edst")
            nc.sync.dma_start(out=edst_sb[:], in_=edst[:])

            # ---------------- dense1 (own shard): p1_own = dinv*(x@W1.T)
            with (
                tc.tile_pool(name="d1x", bufs=3) as xpool,
                tc.tile_pool(name="d1s", bufs=3) as spool,
                tc.tile_pool(name="d1o", bufs=3) as opool,
                tc.tile_pool(name="d1p", bufs=3, space="PSUM") as pspool,
                tc.tile_pool(name="d1t", bufs=4, space="PSUM") as trpool,
            ):
                for r0 in range(0, NSHARD, 512):
                    rw = min(512, NSHARD - r0)
                    xt = xpool.tile([128, 4 * 512], F32, tag="xt")
                    for kc in range(4):
                        nc.sync.dma_start(
                            out=xt[:, kc * 512:kc * 512 + rw],
                            in_=xT_own[kc * 128:(kc + 1) * 128, r0:r0 + rw])
                    t1sb = spool.tile([128, 2 * 512], F32, tag="t1sb")
                    for oc in range(2):
                        ps = pspool.tile([128, 512], F32, space="PSUM", tag="d1ps")
                        for kc in range(4):
                            nc.tensor.matmul(
                                ps[:, :rw],
                                lhsT=w1_sb[:, kc * H + oc * 128:kc * H + (oc + 1) * 128],
                                rhs=xt[:, kc * 512:kc * 512 + rw],
                                start=(kc == 0), stop=(kc == 3))
                        nc.vector.tensor_copy(
                            out=t1sb[:, oc * 512:oc * 512 + rw], in_=ps[:, :rw])
                    for sub in range(rw // 128):
                        t = r0 // 128 + sub
                        p1t = opool.tile([128, H], F32, tag="p1t")
                        for oc in range(2):
                            trp = trpool.tile([128, 128], F32, space="PSUM",
                                              tag="trps")
                            nc.tensor.transpose(
                                trp[:],
                                t1sb[:, oc * 512 + sub * 128:oc * 512 + (sub + 1) * 128],
                                ident[:])
                            nc.vector.tensor_tensor(
                                out=p1t[:, oc * 128:(oc + 1) * 128],
                                in0=trp[:],
                                in1=dinvo_sb[:, t:t + 1].to_broadcast([128, 128]),
                                op=mybir.AluOpType.mult)
                        nc.sync.dma_start(
                            out=p1_own[t * 128:(t + 1) * 128, :], in_=p1t[:])

            nc.gpsimd.collective_compute(
                "AllGather", mybir.AluOpType.bypass, replica_groups=rg,
                ins=[p1_own[:]], outs=[p1_full[:]])
            tc.strict_bb_all_engine_barrier()

            # ---------------- agg1 + dense2 per own tile
            with (
                tc.tile_pool(name="a1g", bufs=24) as gpool,
                tc.tile_pool(name="a1s", bufs=24) as sspool,
                tc.tile_pool(name="a1h", bufs=3) as hpool,
                tc.tile_pool(name="a1o", bufs=3) as opool,
                tc.tile_pool(name="a1p", bufs=2, space="PSUM") as aggps,
                tc.tile_pool(name="a1t", bufs=2, space="PSUM") as trps2,
            ):
                for t in range(TLOC):
                    nb = int(nblk[t])
                    j0 = int(blk_off[t])
                    psum_h = aggps.tile([128, H], F32, space="PSUM", tag="aggps")
                    for b in range(nb):
                        j = j0 + b
                        G = gpool.tile([128, H], F32, tag="G")
                        nc.gpsimd.indirect_dma_start(
                            out=G[:], out_offset=None,
                            in_=p1_full[:],
                            in_offset=bass.IndirectOffsetOnAxis(
                                ap=esrc_sb[:, j:j + 1], axis=0))
                        S = sspool.tile([128, 128], F32, tag="S")
                        nc.vector.tensor_tensor(
                            out=S[:], in0=iota[:],
                            in1=edst_sb[:, j:j + 1].to_broadcast([128, 128]),
                            op=mybir.AluOpType.is_equal)
                        nc.tensor.matmul(psum_h[:], lhsT=S[:], rhs=G[:],
                                         start=(b == 0), stop=(b == nb - 1))
                    hpre = hpool.tile([128, H], F32, tag="hpre")
                    nc.vector.tensor_tensor(
                        out=hpre[:], in0=psum_h[:],
                        in1=dinvo_sb[:, t:t + 1].to_broadcast([128, H]),
                        op=mybir.AluOpType.mult)
                    hT = hpool.tile([128, 256], F32, tag="hT")
                    for oc in range(2):
                        trp = trps2.tile([128, 128], F32, space="PSUM", tag="tr2")
                        nc.tensor.transpose(
                            trp[:], hpre[:, oc * 128:(oc + 1) * 128], ident[:])
                        nc.scalar.activation(
                            out=hT[:, oc * 128:(oc + 1) * 128], in_=trp[:],
                            func=mybir.ActivationFunctionType.Relu,
                            bias=b1_sb[:, oc:oc + 1])
                    t2ps = trps2.tile([128, 128], F32, space="PSUM", tag="t2ps")
                    for oc in range(2):
                        nc.tensor.matmul(
                            t2ps[:],
                            lhsT=wcat_sb[:, oc * 128:(oc + 1) * 128],
                            rhs=hT[:, oc * 128:(oc + 1) * 128],
                            start=(oc == 0), stop=(oc == 1))
                    t2sb = hpool.tile([128, 128], F32, tag="t2sb")
                    nc.vector.tensor_copy(out=t2sb[:], in_=t2ps[:])
                    p2ps = trps2.tile([128, 128], F32, space="PSUM", tag="p2ps")
                    nc.tensor.transpose(p2ps[:], t2sb[:], ident[:])
                    p2t = opool.tile([128, 128], F32, tag="p2t")
                    nc.vector.tensor_tensor(
                        out=p2t[:], in0=p2ps[:],
                        in1=dinvo_sb[:, t:t + 1].to_broadcast([128, 128]),
                        op=mybir.AluOpType.mult)
                    nc.sync.dma_start(
                        out=p2_own[t * 128:(t + 1) * 128, :], in_=p2t[:])

            nc.gpsimd.collective_compute(
                "AllGather", mybir.AluOpType.bypass, replica_groups=rg,
                ins=[p2_own[:]], outs=[p2_full[:]])
            tc.strict_bb_all_engine_barrier()

            # ---------------- agg2 per own tile -> z
            with (
                tc.tile_pool(name="a2g", bufs=24) as gpool,
                tc.tile_pool(name="a2s", bufs=24) as sspool,
                tc.tile_pool(name="a2h", bufs=3) as hpool,
                tc.tile_pool(name="a2e", bufs=3) as epool2,
                tc.tile_pool(name="a2p", bufs=2, space="PSUM") as aggps,
                tc.tile_pool(name="a2t", bufs=2, space="PSUM") as trps3,
            ):
                for t in range(TLOC):
                    nb = int(nblk[t])
                    j0 = int(blk_off[t])
                    psum_a = aggps.tile([128, 128], F32, space="PSUM", tag="agg2ps")
                    for b in range(nb):
                        j = j0 + b
                        G = gpool.tile([128, 128], F32, tag="G2")
                        nc.gpsimd.indirect_dma_start(
                            out=G[:], out_offset=None,
                            in_=p2_full[:],
                            in_offset=bass.IndirectOffsetOnAxis(
                                ap=esrc_sb[:, j:j + 1], axis=0))
                        S = sspool.tile([128, 128], F32, tag="S2")
                        nc.vector.tensor_tensor(
                            out=S[:], in0=iota[:],
                            in1=edst_sb[:, j:j + 1].to_broadcast([128, 128]),
                            op=mybir.AluOpType.is_equal)
                        nc.tensor.matmul(psum_a[:], lhsT=S[:], rhs=G[:],
                                         start=(b == 0), stop=(b == nb - 1))
                    apre = hpool.tile([128, 128], F32, tag="apre")
                    nc.vector.tensor_tensor(
                        out=apre[:], in0=psum_a[:],
                        in1=dinvo_sb[:, t:t + 1].to_broadcast([128, 128]),
                        op=mybir.AluOpType.mult)
                    aTps = trps3.tile([128, 128], F32, space="PSUM", tag="aT")
                    nc.tensor.transpose(aTps[:], apre[:], ident[:])
                    aT = hpool.tile([128, 128], F32, tag="aTsb")
                    nc.vector.tensor_tensor(
                        out=aT[:], in0=aTps[:],
                        in1=bcat_sb[:, 0:1].to_broadcast([128, 128]),
                        op=mybir.AluOpType.add)
                    nc.vector.tensor_tensor(
                        out=aT[64:128, :], in0=aT[64:128, :], in1=c10[64:128, :],
                        op=mybir.AluOpType.min)
                    std = hpool.tile([64, 128], F32, tag="std")
                    nc.scalar.activation(
                        out=std[:], in_=aT[64:128, :],
                        func=mybir.ActivationFunctionType.Exp)
                    et = epool2.tile([64, 128], F32, tag="et")
                    nc.sync.dma_start(out=et[:],
                                      in_=epsT[:, t * 128:(t + 1) * 128])
                    zT = zT_all[:, t * 128:(t + 1) * 128]
                    nc.vector.tensor_tensor(out=zT, in0=std[:], in1=et[:],
                                            op=mybir.AluOpType.mult)
                    nc.vector.tensor_tensor(out=zT, in0=zT, in1=aT[:64, :],
                                            op=mybir.AluOpType.add)
                    zps = trps3.tile([128, 64], F32, space="PSUM", tag="zps")
                    nc.tensor.transpose(zps[:], zT, ident[:64, :64])
                    zt = hpool.tile([128, 64], F32, tag="zt")
                    nc.vector.tensor_copy(out=zt[:], in_=zps[:])
                    nc.sync.dma_start(out=z_own[t * 128:(t + 1) * 128, :],
                                      in_=zt[:])
                    nc.sync.dma_start(out=z_out[t * 128:(t + 1) * 128, :],
                                      in_=zt[:])

            nc.gpsimd.collective_compute(
                "AllGather", mybir.AluOpType.bypass, replica_groups=rg,
                ins=[z_own[:]], outs=[z_full[:]])
            tc.strict_bb_all_engine_barrier()

            # ---------------- scoring: s = sigmoid(z[u].z[v]), u in own shard
            with (
                tc.tile_pool(name="scg", bufs=16) as gpool,
                tc.tile_pool(name="scw", bufs=10) as wpool,
                tc.tile_pool(name="sco", bufs=2) as spool3,
                tc.tile_pool(name="scp", bufs=4, space="PSUM") as scps,
                tc.tile_pool(name="sct", bufs=4, space="PSUM") as sctr,
            ):
                su_sb = epool.tile([128, NSB2], F32, tag="su")
                nc.sync.dma_start(out=su_sb[:], in_=su[:])
                sv_sb = epool.tile([128, NSB2], I32, tag="sv")
                nc.sync.dma_start(out=sv_sb[:], in_=sv[:])
                PK = 8
                pack = None
                NSBTOT = int(nblkS.sum())
                for t in range(TLOC):
                    nbs = int(nblkS[t])
                    g0 = int(blk_offS[t])
                    for b in range(nbs):
                        g = g0 + b
                        if g % PK == 0:
                            pack = spool3.tile([128, PK], F32, tag="pack")
                        Gv = gpool.tile([128, DZ], F32, tag="Gv")
                        nc.gpsimd.indirect_dma_start(
                            out=Gv[:], out_offset=None, in_=z_full[:],
                            in_offset=bass.IndirectOffsetOnAxis(
                                ap=sv_sb[:, g:g + 1], axis=0))
                        vtp = sctr.tile([64, 128], F32, space="PSUM", tag="vtp")
                        nc.tensor.transpose(vtp[:], Gv[:], ident[:])
                        ZvT = wpool.tile([64, 128], F32, tag="ZvT")
                        nc.vector.tensor_copy(out=ZvT[:], in_=vtp[:])
                        Pps = scps.tile([128, 128], F32, space="PSUM", tag="Pps")
                        nc.tensor.matmul(
                            Pps[:], lhsT=ZvT[:],
                            rhs=zT_all[:, t * 128:(t + 1) * 128],
                            start=True, stop=True)
                        S = wpool.tile([128, 128], F32, tag="Sm")
                        nc.vector.tensor_tensor(
                            out=S[:], in0=iota[:],
                            in1=su_sb[:, g:g + 1].to_broadcast([128, 128]),
                            op=mybir.AluOpType.is_equal)
                        m = wpool.tile([128, 128], F32, tag="m")
                        nc.vector.tensor_tensor(out=m[:], in0=Pps[:], in1=S[:],
                                                op=mybir.AluOpType.mult)
                        r = wpool.tile([128, 1], F32, tag="r")
                        nc.vector.reduce_sum(r[:], m[:],
                                             axis=mybir.AxisListType.X)
                        nc.scalar.activation(
                            out=pack[:, g % PK:g % PK + 1], in_=r[:],
                            func=mybir.ActivationFunctionType.Sigmoid)
                        if (g % PK == PK - 1) or (g == NSBTOT - 1):
                            gs = (g // PK) * PK
                            gw = g - gs + 1
                            nc.sync.dma_start(
                                out=s_out[gs * 128:(gs + gw) * 128].rearrange(
                                    "(j p) -> p j", p=128),
                                in_=pack[:, :gw])

    nc.compile()
    return nc


_CACHE = {}


def kernel(x, edge_index, target_edge_index, W1, b1, W_mu, b_mu, W_ls, b_ls,
           eps_noise):
    in_maps, nblk, NBLK, nblkS, NSB2, meta = _host_prep(
        x, edge_index, target_edge_index, W1, b1, W_mu, b_mu, W_ls, b_ls,
        eps_noise)
    key = (NBLK, NSB2, tuple(nblk.tolist()), tuple(nblkS.tolist()))
    if key not in _CACHE:
        _CACHE[key] = _build(nblk, NBLK, nblkS, NSB2)
    nc = _CACHE[key]

    trace = _HAVE_HOOK and os.environ.get("KERNEL_NO_TRACE", "0") != "1"
    try:
        res = run_bass_kernel_spmd(nc, in_maps, list(range(NCORE)), trace=trace)
    except Exception:
        if not trace:
            raise
        res = run_bass_kernel_spmd(nc, in_maps, list(range(NCORE)), trace=False)
    kernel.last_exec_time_ns = getattr(res, "exec_time_ns", None)

    perm = meta["perm"]
    sslot = meta["sslot"]
    z_dev = np.concatenate([res.results[c]["z_out"] for c in range(NCORE)],
                           axis=0)
    z = z_dev[perm[:N]]
    s = np.empty(ET, np.float32)
    for c in range(NCORE):
        sl = sslot[c]
        valid = sl >= 0
        s[sl[valid]] = res.results[c]["s_out"][valid]
    return z, s


kernel.last_exec_time_ns = None
